# revision 3
# baseline (speedup 1.0000x reference)
"""SEIR physics PINN recurrence on 8 Trainium2 NeuronCores.

Strategy (data-parallel over batch, 16 rows/core):
  - Pi = softmax(pi_logits) computed on-device; beta folded into the matmul
    weights: PiB = Pi * beta[col] cast to bf16 (stationary operand, FWL).
  - Two-step batching: I_{t+1} = relu((1-g)I_t + s*E_t) does not depend on
    lam_t, so lam for two consecutive steps is computed by one pass over Pi
    (rhs free dim 32 instead of 16) -> half the LDWEIGHTS traffic.
  - Matmul emits lam^T directly (N-on-partition), matching the elementwise
    state layout [128 part = n%128, free = (n//128, batch)]. No transposes.
  - States E,S,I kept fp32 (bf16 state feedback loses too much precision);
    only the matmul operands are bf16.  -S is stored so the S-clamp is a
    single two-op tensor_scalar (min, subtract).
  - Elementwise split into two j-chunk halves so state updates for half 0
    overlap the PE's matmuls that only need half 1, closing the
    PE->DVE->PE serial chain into a pipeline.
Output layout on device: i_out[t, n, b] (b innermost, 64B bursts); host
transposes to [b, t, n] and concatenates core shards.
"""
import sys

sys.path.insert(0, "/opt/trn_rl_repo")

import numpy as np
import ml_dtypes

import concourse.bass as bass
import concourse.mybir as mybir
from concourse import bacc
from concourse.tile import TileContext
from concourse.bass_utils import run_bass_kernel_spmd

bf16 = ml_dtypes.bfloat16
F32 = mybir.dt.float32
BF16 = mybir.dt.bfloat16
Alu = mybir.AluOpType
Act = mybir.ActivationFunctionType

B, T, N = 128, 256, 1024
NCORES = 8
BL = B // NCORES          # 16 batch rows per core
NJ = N // 128             # 8 n-chunks of 128
PAIRS = (T - 2) // 2      # 127 two-step iterations
HJ = NJ // 2              # chunks per half

_cache = {}


def _build(c1s: float, sg: float, c1g: float, beta: float):
    """Build the SPMD program. c1s=1-sigma, sg=sigma, c1g=1-gamma."""
    nc = bacc.Bacc("TRN2", target_bir_lowering=False, debug=False,
                   num_devices=NCORES)
    logits = nc.dram_tensor("logits", [N, N], F32, kind="ExternalInput").ap()
    x0 = nc.dram_tensor("x0", [128, NJ, BL], F32, kind="ExternalInput").ap()
    i_out = nc.dram_tensor("i_out", [T, N, BL], F32, kind="ExternalOutput").ap()
    pi_out = nc.dram_tensor("pi_out", [N, N], F32, kind="ExternalOutput").ap()

    with TileContext(nc) as tc:
        with tc.tile_pool(name="pib", bufs=1) as pibp, \
             tc.tile_pool(name="soft", bufs=2) as soft, \
             tc.tile_pool(name="lp", bufs=1) as lp, \
             tc.tile_pool(name="ps", bufs=2, space="PSUM") as psp:

            # ---- Phase 1: softmax + beta-fold (PiB weights, bf16) ----
            pib = []
            for k in range(NJ):
                lg = soft.tile([128, N], F32, tag="lg", bufs=2)
                nc.sync.dma_start(out=lg, in_=logits[k * 128:(k + 1) * 128, :])
                ex = soft.tile([128, N], F32, tag="ex", bufs=2)
                nc.scalar.activation(ex, lg, Act.Exp)
                sm = soft.tile([128, 1], F32, tag="sm", bufs=2)
                nc.vector.tensor_reduce(sm, ex, axis=mybir.AxisListType.X,
                                        op=Alu.add)
                rc = soft.tile([128, 1], F32, tag="rc", bufs=2)
                nc.vector.reciprocal(rc, sm)
                pi = soft.tile([128, N], F32, tag="pi", bufs=2)
                nc.vector.tensor_scalar(pi, ex, rc, None, Alu.mult)
                nc.sync.dma_start(out=pi_out[k * 128:(k + 1) * 128, :], in_=pi)
                pb = pibp.tile([128, N], BF16, tag=f"pib{k}")
                nc.vector.tensor_scalar(pb, pi, float(beta), None, Alu.mult)
                pib.append(pb)

            # ---- Phase 2: initial states, per half ----
            # E0 = 0, S0n = -0.99, I0 = relu(x0), I1 = relu((1-g) I0)
            E = [None, None]    # fp32 [128, HJ, BL], E_{2p}
            Sn = [None, None]   # fp32, NEGATED S_{2p}
            If = [None, None]   # fp32, I_{2p+1}
            Ip = [None, None]   # bf16 [128, HJ, 2, BL], (I_{2p}, I_{2p+1})
            for h in range(2):
                jlo = h * HJ
                x0t = lp.tile([128, HJ, BL], F32, tag=f"x0{h}", bufs=2)
                nc.sync.dma_start(out=x0t, in_=x0[:, jlo:jlo + HJ, :])
                e0 = lp.tile([128, HJ, BL], F32, tag=f"E{h}", bufs=4)
                nc.vector.memset(e0, 0.0)
                s0 = lp.tile([128, HJ, BL], F32, tag=f"S{h}", bufs=4)
                nc.vector.memset(s0, -0.99)
                i0 = lp.tile([128, HJ, BL], F32, tag=f"I{h}", bufs=4)
                nc.vector.tensor_scalar(i0, x0t, 0.0, None, Alu.max)
                i1 = lp.tile([128, HJ, BL], F32, tag=f"I{h}", bufs=4)
                nc.scalar.activation(i1, i0, Act.Relu, scale=float(c1g))
                ip = lp.tile([128, HJ, 2, BL], BF16, tag=f"P{h}", bufs=3)
                nc.vector.tensor_copy(ip[:, :, 0, :], i0)
                nc.vector.tensor_copy(ip[:, :, 1, :], i1)
                dst = i_out[0].rearrange("(j p) b -> p j b", p=128)
                nc.sync.dma_start(out=dst[:, jlo:jlo + HJ, :], in_=i0)
                dst = i_out[1].rearrange("(j p) b -> p j b", p=128)
                nc.sync.dma_start(out=dst[:, jlo:jlo + HJ, :], in_=i1)
                E[h], Sn[h], If[h], Ip[h] = e0, s0, i1, ip

            # ---- Phase 3: pair loop ----
            for p in range(PAIRS):
                t2 = 2 * p + 2
                ps = [psp.tile([128, HJ, 2, BL], F32, tag=f"ps{h}", bufs=2,
                               name=f"ps{h}_{p}")
                      for h in range(2)]
                # matmul sweeps: (j-half, k-half) in order A,B,C,D.
                for hj in range(2):
                    for kh in range(2):
                        for jj in range(HJ):
                            j = hj * HJ + jj
                            for kk in range(HJ):
                                k = kh * HJ + kk
                                # start=True lazily zeroes the WHOLE 2KB psum
                                # bank, so only the first matmul per psum
                                # tile may carry it; all other groups
                                # accumulate onto the lazily-zeroed bytes.
                                nc.tensor.matmul(
                                    ps[hj][:, jj, :, :],
                                    pib[k][:, j * 128:(j + 1) * 128],
                                    Ip[kh][:, kk, :, :],
                                    start=(kh == 0 and jj == 0 and kk == 0),
                                    stop=(k == NJ - 1),
                                    skip_group_check=True)
                # elementwise per half
                for h in range(2):
                    jlo = h * HJ
                    Ecur, Scur, Icur, Pcur = E[h], Sn[h], If[h], Ip[h]
                    Pnext = lp.tile([128, HJ, 2, BL], BF16, tag=f"P{h}", bufs=3)
                    Ifs = []
                    for s in range(2):
                        # E' = (1-sg)*Ecur + S*lamB  (Scur negated)
                        Mh = lp.tile([128, HJ, BL], F32, tag=f"M{h}", bufs=2)
                        nc.vector.tensor_mul(Mh, Scur, ps[h][:, :, s, :])
                        A0 = lp.tile([128, HJ, BL], F32, tag=f"A{h}", bufs=2)
                        nc.scalar.activation(A0, Ecur, Act.Copy, scale=float(c1s))
                        E1 = lp.tile([128, HJ, BL], F32, tag=f"E{h}", bufs=4)
                        nc.vector.tensor_sub(E1, A0, Mh)
                        # S' = max(1 - (E'+I'), .01); stored negated:
                        # Sn' = min(E'+I', .99) - 1
                        C0 = lp.tile([128, HJ, BL], F32, tag=f"C{h}", bufs=2)
                        nc.vector.tensor_add(C0, E1, Icur)
                        S1 = lp.tile([128, HJ, BL], F32, tag=f"S{h}", bufs=4)
                        nc.vector.tensor_scalar(S1, C0, 0.99, 1.0,
                                                Alu.min, Alu.subtract)
                        # I'' = relu((1-g)*I' + sg*E')
                        B0 = lp.tile([128, HJ, BL], F32, tag=f"B{h}", bufs=2)
                        nc.vector.tensor_scalar(B0, E1, float(sg), None,
                                                Alu.mult)
                        Ipre = lp.tile([128, HJ, BL], F32, tag=f"R{h}", bufs=2)
                        nc.scalar.activation(Ipre, Icur, Act.Copy,
                                             scale=float(c1g))
                        Isum = lp.tile([128, HJ, BL], F32, tag=f"U{h}", bufs=2)
                        nc.vector.tensor_add(Isum, Ipre, B0)
                        I2 = lp.tile([128, HJ, BL], F32, tag=f"I{h}", bufs=4)
                        nc.scalar.activation(I2, Isum, Act.Relu)
                        nc.vector.tensor_copy(Pnext[:, :, s, :], I2)
                        dst = i_out[t2 + s].rearrange("(j p) b -> p j b", p=128)
                        nc.sync.dma_start(out=dst[:, jlo:jlo + HJ, :], in_=I2)
                        Ecur, Scur, Icur = E1, S1, I2
                        Ifs.append(I2)
                    E[h], Sn[h], If[h], Ip[h] = Ecur, Scur, Ifs[1], Pnext
    nc.finalize()
    return nc


def _get_program(c1s, sg, c1g, beta):
    key = (round(c1s, 9), round(sg, 9), round(c1g, 9), round(beta, 9))
    if key not in _cache:
        _cache[key] = _build(c1s, sg, c1g, beta)
    return _cache[key]


def _run(x_hist, beta, sigma, gamma, pi_logits, trace=False):
    assert x_hist.shape == (B, T, N) and pi_logits.shape == (N, N)
    for v, nm in ((beta, "beta"), (sigma, "sigma"), (gamma, "gamma")):
        assert np.ptp(np.asarray(v)) == 0.0, f"{nm} must be uniform"
    bta = float(np.asarray(beta).flat[0])
    sgm = float(np.asarray(sigma).flat[0])
    gma = float(np.asarray(gamma).flat[0])

    nc = _get_program(1.0 - sgm, sgm, 1.0 - gma, bta)

    lg = np.ascontiguousarray(np.asarray(pi_logits, dtype=np.float32))
    in_maps = []
    for c in range(NCORES):
        x0s = np.asarray(x_hist[c * BL:(c + 1) * BL, 0, :], dtype=np.float32)
        # pack [p, j, b]: x0p[p, j, b] = x0s[b, j*128+p]
        x0p = np.ascontiguousarray(
            np.transpose(x0s.reshape(BL, NJ, 128), (2, 1, 0)))
        in_maps.append({"logits": lg, "x0": x0p})

    res = run_bass_kernel_spmd(nc, in_maps, list(range(NCORES)), trace=trace)
    shards = []
    for c in range(NCORES):
        o = res.results[c]["i_out"]          # [T, N, BL]
        shards.append(np.transpose(o, (2, 0, 1)))   # [BL, T, N]
    i_sim = np.ascontiguousarray(np.concatenate(shards, axis=0))
    pi = res.results[0]["pi_out"]
    return (i_sim, pi), res


def kernel(x_hist, beta, sigma, gamma, pi_logits):
    out, _ = _run(x_hist, beta, sigma, gamma, pi_logits)
    return out


# revision 4
# speedup vs baseline: 8.2991x; 8.2991x over previous
"""SEIR physics PINN recurrence on 8 Trainium2 NeuronCores.

Strategy (data-parallel over batch, 16 rows/core):
  - Pi = softmax(pi_logits) computed on-device; beta folded into the matmul
    weights: PiB = Pi * beta[col] cast to bf16 (stationary operand, FWL).
  - Two-step batching: I_{t+1} = relu((1-g)I_t + s*E_t) does not depend on
    lam_t, so lam for two consecutive steps is computed by one pass over Pi
    (rhs free dim 32 instead of 16) -> half the LDWEIGHTS traffic.
  - Matmul emits lam^T directly (N-on-partition), matching the elementwise
    state layout [128 part = n%128, free = (n//128, batch)]. No transposes.
  - States E,S,I kept fp32 (bf16 state feedback loses too much precision);
    only the matmul operands are bf16.  -S is stored so the S-clamp is a
    single two-op tensor_scalar (min, subtract).
  - Elementwise split into two j-chunk halves so state updates for half 0
    overlap the PE's matmuls that only need half 1, closing the
    PE->DVE->PE serial chain into a pipeline.
Output layout on device: i_out[t, n, b] (b innermost, 64B bursts); host
transposes to [b, t, n] and concatenates core shards.
"""
import sys

sys.path.insert(0, "/opt/trn_rl_repo")

import numpy as np
import ml_dtypes

import concourse.bass as bass
import concourse.mybir as mybir
from concourse import bacc
from concourse.tile import TileContext
from concourse.bass_utils import run_bass_kernel_spmd

bf16 = ml_dtypes.bfloat16
F32 = mybir.dt.float32
BF16 = mybir.dt.bfloat16
Alu = mybir.AluOpType
Act = mybir.ActivationFunctionType

B, T, N = 128, 256, 1024
NCORES = 8
BL = B // NCORES          # 16 batch rows per core
NJ = N // 128             # 8 n-chunks of 128
PAIRS = (T - 2) // 2      # 127 two-step iterations
HJ = NJ // 2              # chunks per half

_cache = {}


def _build(c1s: float, sg: float, c1g: float, beta: float, npairs: int = PAIRS, out_dmas: bool = True):
    """Build the SPMD program. c1s=1-sigma, sg=sigma, c1g=1-gamma."""
    nc = bacc.Bacc("TRN2", target_bir_lowering=False, debug=False,
                   num_devices=NCORES)
    logits = nc.dram_tensor("logits", [N, N], F32, kind="ExternalInput").ap()
    x0 = nc.dram_tensor("x0", [128, NJ, BL], F32, kind="ExternalInput").ap()
    i_out = nc.dram_tensor("i_out", [T, N, BL], F32, kind="ExternalOutput").ap()
    pi_out = nc.dram_tensor("pi_out", [N, N], F32, kind="ExternalOutput").ap()

    with TileContext(nc) as tc:
        with tc.tile_pool(name="pib", bufs=1) as pibp, \
             tc.tile_pool(name="soft", bufs=2) as soft, \
             tc.tile_pool(name="lp", bufs=1) as lp, \
             tc.tile_pool(name="ps", bufs=2, space="PSUM") as psp:

            # ---- Phase 1: softmax + beta-fold (PiB weights, bf16) ----
            pib = []
            for k in range(NJ):
                lg = soft.tile([128, N], F32, tag="lg", bufs=2)
                nc.sync.dma_start(out=lg, in_=logits[k * 128:(k + 1) * 128, :])
                ex = soft.tile([128, N], F32, tag="ex", bufs=2)
                nc.scalar.activation(ex, lg, Act.Exp)
                sm = soft.tile([128, 1], F32, tag="sm", bufs=2)
                nc.vector.tensor_reduce(sm, ex, axis=mybir.AxisListType.X,
                                        op=Alu.add)
                rc = soft.tile([128, 1], F32, tag="rc", bufs=2)
                nc.vector.reciprocal(rc, sm)
                pi = soft.tile([128, N], F32, tag="pi", bufs=2)
                nc.vector.tensor_scalar(pi, ex, rc, None, Alu.mult)
                nc.sync.dma_start(out=pi_out[k * 128:(k + 1) * 128, :], in_=pi)
                pb = pibp.tile([128, N], BF16, tag=f"pib{k}")
                nc.vector.tensor_scalar(pb, pi, float(beta), None, Alu.mult)
                pib.append(pb)

            # ---- Phase 2: initial states, per half ----
            # E0 = 0, S0n = -0.99, I0 = relu(x0), I1 = relu((1-g) I0)
            E = [None, None]    # fp32 [128, HJ, BL], E_{2p}
            Sn = [None, None]   # fp32, NEGATED S_{2p}
            If = [None, None]   # fp32, I_{2p+1}
            Ip = [None, None]   # bf16 [128, HJ, 2, BL], (I_{2p}, I_{2p+1})
            for h in range(2):
                jlo = h * HJ
                x0t = lp.tile([128, HJ, BL], F32, tag=f"x0{h}", bufs=2)
                nc.sync.dma_start(out=x0t, in_=x0[:, jlo:jlo + HJ, :])
                e0 = lp.tile([128, HJ, BL], F32, tag=f"E{h}", bufs=4)
                nc.vector.memset(e0, 0.0)
                s0 = lp.tile([128, HJ, BL], F32, tag=f"S{h}", bufs=4)
                nc.vector.memset(s0, -0.99)
                i0 = lp.tile([128, HJ, BL], F32, tag=f"I{h}", bufs=4)
                nc.vector.tensor_scalar(i0, x0t, 0.0, None, Alu.max)
                i1 = lp.tile([128, HJ, BL], F32, tag=f"I{h}", bufs=4)
                nc.scalar.activation(i1, i0, Act.Relu, scale=float(c1g))
                ip = lp.tile([128, HJ, 2, BL], BF16, tag=f"P{h}", bufs=3)
                nc.vector.tensor_copy(ip[:, :, 0, :], i0)
                nc.vector.tensor_copy(ip[:, :, 1, :], i1)
                dst = i_out[0].rearrange("(j p) b -> p j b", p=128)
                nc.sync.dma_start(out=dst[:, jlo:jlo + HJ, :], in_=i0)
                dst = i_out[1].rearrange("(j p) b -> p j b", p=128)
                nc.sync.dma_start(out=dst[:, jlo:jlo + HJ, :], in_=i1)
                E[h], Sn[h], If[h], Ip[h] = e0, s0, i1, ip

            # ---- Phase 3: pair loop ----
            for p in range(npairs):
                t2 = 2 * p + 2
                ps = [psp.tile([128, HJ, 2, BL], F32, tag=f"ps{h}", bufs=2,
                               name=f"ps{h}_{p}")
                      for h in range(2)]
                # matmul sweeps: (j-half, k-half) in order A,B,C,D.
                for hj in range(2):
                    for kh in range(2):
                        for jj in range(HJ):
                            j = hj * HJ + jj
                            for kk in range(HJ):
                                k = kh * HJ + kk
                                # start=True lazily zeroes the WHOLE 2KB psum
                                # bank, so only the first matmul per psum
                                # tile may carry it; all other groups
                                # accumulate onto the lazily-zeroed bytes.
                                nc.tensor.matmul(
                                    ps[hj][:, jj, :, :],
                                    pib[k][:, j * 128:(j + 1) * 128],
                                    Ip[kh][:, kk, :, :],
                                    start=(kh == 0 and jj == 0 and kk == 0),
                                    stop=(k == NJ - 1),
                                    skip_group_check=True)
                # elementwise per half
                for h in range(2):
                    jlo = h * HJ
                    Ecur, Scur, Icur, Pcur = E[h], Sn[h], If[h], Ip[h]
                    Pnext = lp.tile([128, HJ, 2, BL], BF16, tag=f"P{h}", bufs=3)
                    Ifs = []
                    for s in range(2):
                        # E' = (1-sg)*Ecur + S*lamB  (Scur negated)
                        Mh = lp.tile([128, HJ, BL], F32, tag=f"M{h}", bufs=2)
                        nc.vector.tensor_mul(Mh, Scur, ps[h][:, :, s, :])
                        A0 = lp.tile([128, HJ, BL], F32, tag=f"A{h}", bufs=2)
                        nc.scalar.activation(A0, Ecur, Act.Copy, scale=float(c1s))
                        E1 = lp.tile([128, HJ, BL], F32, tag=f"E{h}", bufs=4)
                        nc.vector.tensor_sub(E1, A0, Mh)
                        # S' = max(1 - (E'+I'), .01); stored negated:
                        # Sn' = min(E'+I', .99) - 1
                        C0 = lp.tile([128, HJ, BL], F32, tag=f"C{h}", bufs=2)
                        nc.vector.tensor_add(C0, E1, Icur)
                        S1 = lp.tile([128, HJ, BL], F32, tag=f"S{h}", bufs=4)
                        nc.vector.tensor_scalar(S1, C0, 0.99, 1.0,
                                                Alu.min, Alu.subtract)
                        # I'' = relu((1-g)*I' + sg*E')
                        B0 = lp.tile([128, HJ, BL], F32, tag=f"B{h}", bufs=2)
                        nc.vector.tensor_scalar(B0, E1, float(sg), None,
                                                Alu.mult)
                        Ipre = lp.tile([128, HJ, BL], F32, tag=f"R{h}", bufs=2)
                        nc.scalar.activation(Ipre, Icur, Act.Copy,
                                             scale=float(c1g))
                        Isum = lp.tile([128, HJ, BL], F32, tag=f"U{h}", bufs=2)
                        nc.vector.tensor_add(Isum, Ipre, B0)
                        I2 = lp.tile([128, HJ, BL], F32, tag=f"I{h}", bufs=4)
                        nc.scalar.activation(I2, Isum, Act.Relu)
                        nc.vector.tensor_copy(Pnext[:, :, s, :], I2)
                        if out_dmas:
                            dst = i_out[t2 + s].rearrange("(j p) b -> p j b", p=128)
                            nc.sync.dma_start(out=dst[:, jlo:jlo + HJ, :], in_=I2)
                        Ecur, Scur, Icur = E1, S1, I2
                        Ifs.append(I2)
                    E[h], Sn[h], If[h], Ip[h] = Ecur, Scur, Ifs[1], Pnext
    nc.finalize()
    return nc


def _get_program(c1s, sg, c1g, beta, npairs=PAIRS, out_dmas=True):
    key = (round(c1s, 9), round(sg, 9), round(c1g, 9), round(beta, 9), npairs, out_dmas)
    if key not in _cache:
        _cache[key] = _build(c1s, sg, c1g, beta, npairs, out_dmas)
    return _cache[key]


def _run(x_hist, beta, sigma, gamma, pi_logits, trace=False):
    assert x_hist.shape == (B, T, N) and pi_logits.shape == (N, N)
    for v, nm in ((beta, "beta"), (sigma, "sigma"), (gamma, "gamma")):
        assert np.ptp(np.asarray(v)) == 0.0, f"{nm} must be uniform"
    bta = float(np.asarray(beta).flat[0])
    sgm = float(np.asarray(sigma).flat[0])
    gma = float(np.asarray(gamma).flat[0])

    nc = _get_program(1.0 - sgm, sgm, 1.0 - gma, bta)

    lg = np.ascontiguousarray(np.asarray(pi_logits, dtype=np.float32))
    in_maps = []
    for c in range(NCORES):
        x0s = np.asarray(x_hist[c * BL:(c + 1) * BL, 0, :], dtype=np.float32)
        # pack [p, j, b]: x0p[p, j, b] = x0s[b, j*128+p]
        x0p = np.ascontiguousarray(
            np.transpose(x0s.reshape(BL, NJ, 128), (2, 1, 0)))
        in_maps.append({"logits": lg, "x0": x0p})

    res = run_bass_kernel_spmd(nc, in_maps, list(range(NCORES)), trace=trace)
    shards = []
    for c in range(NCORES):
        o = res.results[c]["i_out"]          # [T, N, BL]
        shards.append(np.transpose(o, (2, 0, 1)))   # [BL, T, N]
    i_sim = np.ascontiguousarray(np.concatenate(shards, axis=0))
    pi = res.results[0]["pi_out"]
    return (i_sim, pi), res


def kernel(x_hist, beta, sigma, gamma, pi_logits):
    out, _ = _run(x_hist, beta, sigma, gamma, pi_logits)
    return out


# revision 8
# speedup vs baseline: 175.2652x; 21.1186x over previous
"""SEIR physics PINN recurrence on 8 Trainium2 NeuronCores.

Strategy (data-parallel over batch, 16 rows/core):
  - Pi = softmax(pi_logits) computed on-device; beta folded into the matmul
    weights: PiB = Pi * beta[col] cast to bf16 (stationary operand, FWL).
  - Two-step batching: I_{t+1} = relu((1-g)I_t + s*E_t) does not depend on
    lam_t, so lam for two consecutive steps is computed by one pass over Pi
    (rhs free dim 32 instead of 16) -> half the LDWEIGHTS traffic.
  - Matmul emits lam^T directly (N-on-partition), matching the elementwise
    state layout [128 part = n%128, free = (n//128, batch)]. No transposes.
  - States E,S,I kept fp32 (bf16 state feedback loses too much precision);
    only the matmul operands are bf16.  -S is stored so the S-clamp is a
    single two-op tensor_scalar (min, subtract).
  - Elementwise split into two j-chunk halves so state updates for half 0
    overlap the PE's matmuls that only need half 1, closing the
    PE->DVE->PE serial chain into a pipeline.
Output layout on device: i_out[t, n, b] (b innermost, 64B bursts); host
transposes to [b, t, n] and concatenates core shards.
"""
import sys

sys.path.insert(0, "/opt/trn_rl_repo")

import numpy as np
import ml_dtypes

import concourse.bass as bass
import concourse.mybir as mybir
from concourse import bacc
from concourse.tile import TileContext
from concourse.bass_utils import run_bass_kernel_spmd

bf16 = ml_dtypes.bfloat16
F32 = mybir.dt.float32
BF16 = mybir.dt.bfloat16
Alu = mybir.AluOpType
Act = mybir.ActivationFunctionType

B, T, N = 128, 256, 1024
NCORES = 8
BL = B // NCORES          # 16 batch rows per core
NJ = N // 128             # 8 n-chunks of 128
PAIRS = (T - 2) // 2      # 127 two-step iterations
HJ = NJ // 2              # chunks per half

_cache = {}


def _build(c1s: float, sg: float, c1g: float, beta: float, npairs: int = PAIRS, out_dmas: bool = True, skip_ew: bool = False):
    """Build the SPMD program. c1s=1-sigma, sg=sigma, c1g=1-gamma."""
    nc = bacc.Bacc("TRN2", target_bir_lowering=False, debug=False,
                   num_devices=NCORES)
    logits = nc.dram_tensor("logits", [N, N], F32, kind="ExternalInput").ap()
    x0 = nc.dram_tensor("x0", [128, NJ, BL], F32, kind="ExternalInput").ap()
    i_out = nc.dram_tensor("i_out", [T, 128, NJ, BL], F32, kind="ExternalOutput").ap()
    pi_out = nc.dram_tensor("pi_out", [N, N], F32, kind="ExternalOutput").ap()

    with TileContext(nc) as tc:
        with tc.tile_pool(name="pib", bufs=1) as pibp, \
             tc.tile_pool(name="soft", bufs=2) as soft, \
             tc.tile_pool(name="lp", bufs=1) as lp, \
             tc.tile_pool(name="ps", bufs=3, space="PSUM") as psp:

            # ---- Phase 1: softmax + beta-fold (PiB weights, bf16) ----
            pib = []
            for k in range(NJ):
                lg = soft.tile([128, N], F32, tag="lg", bufs=2)
                nc.sync.dma_start(out=lg, in_=logits[k * 128:(k + 1) * 128, :])
                ex = soft.tile([128, N], F32, tag="ex", bufs=2)
                nc.scalar.activation(ex, lg, Act.Exp)
                sm = soft.tile([128, 1], F32, tag="sm", bufs=2)
                nc.vector.tensor_reduce(sm, ex, axis=mybir.AxisListType.X,
                                        op=Alu.add)
                rc = soft.tile([128, 1], F32, tag="rc", bufs=2)
                nc.vector.reciprocal(rc, sm)
                pi = soft.tile([128, N], F32, tag="pi", bufs=2)
                nc.vector.tensor_scalar(pi, ex, rc, None, Alu.mult)
                nc.sync.dma_start(out=pi_out[k * 128:(k + 1) * 128, :], in_=pi)
                pb = pibp.tile([128, N], BF16, tag=f"pib{k}")
                nc.vector.tensor_scalar(pb, pi, float(beta), None, Alu.mult)
                pib.append(pb)

            # ---- Phase 2: initial states, per half ----
            # E0 = 0, S0n = -0.99, I0 = relu(x0), I1 = relu((1-g) I0)
            E = [None, None]    # fp32 [128, HJ, BL], E_{2p}
            Sn = [None, None]   # fp32, NEGATED S_{2p}
            If = [None, None]   # fp32, I_{2p+1}
            Ip = [None, None]   # bf16 [128, HJ, 2, BL], (I_{2p}, I_{2p+1})
            for h in range(2):
                jlo = h * HJ
                x0t = lp.tile([128, HJ, BL], F32, tag=f"x0{h}", bufs=2)
                nc.sync.dma_start(out=x0t, in_=x0[:, jlo:jlo + HJ, :])
                e0 = lp.tile([128, HJ, BL], F32, tag=f"E{h}", bufs=4)
                nc.vector.memset(e0, 0.0)
                s0 = lp.tile([128, HJ, BL], F32, tag=f"S{h}", bufs=4)
                nc.vector.memset(s0, -0.99)
                i0 = lp.tile([128, HJ, BL], F32, tag=f"I{h}", bufs=4)
                nc.vector.tensor_scalar(i0, x0t, 0.0, None, Alu.max)
                i1 = lp.tile([128, HJ, BL], F32, tag=f"I{h}", bufs=4)
                nc.scalar.activation(i1, i0, Act.Relu, scale=float(c1g))
                ip = lp.tile([128, HJ, 2, BL], BF16, tag=f"P{h}", bufs=4)
                nc.vector.tensor_copy(ip[:, :, 0, :], i0)
                nc.vector.tensor_copy(ip[:, :, 1, :], i1)
                nc.sync.dma_start(out=i_out[0, :, jlo:jlo + HJ, :], in_=i0)
                nc.sync.dma_start(out=i_out[1, :, jlo:jlo + HJ, :], in_=i1)
                E[h], Sn[h], If[h], Ip[h] = e0, s0, i1, ip

            # ---- Phase 3: pair loop ----
            for p in range(npairs):
                t2 = 2 * (p % PAIRS) + 2
                ps = [psp.tile([128, HJ, 2, BL], F32, tag=f"ps{h}", bufs=3,
                               name=f"ps{h}_{p}")
                      for h in range(2)]
                # matmul sweeps: (j-half, k-half); elementwise for j-half 0
                # emitted between the sweep halves as a scheduling hint.
                # Snapshot the rhs tiles: elementwise(0) reassigns Ip[0] to
                # the next pair's tile before sweeps(1) is emitted.
                Ipc = list(Ip)

                def sweeps(hj):
                    for kh in range(2):
                        for jj in range(HJ):
                            j = hj * HJ + jj
                            for kk in range(HJ):
                                k = kh * HJ + kk
                                # start=True lazily zeroes the WHOLE 2KB psum
                                # bank, so only the first matmul per psum
                                # tile may carry it; all other groups
                                # accumulate onto the lazily-zeroed bytes.
                                nc.tensor.matmul(
                                    ps[hj][:, jj, :, :],
                                    pib[k][:, j * 128:(j + 1) * 128],
                                    Ipc[kh][:, kk, :, :],
                                    start=(kh == 0 and jj == 0 and kk == 0),
                                    stop=(k == NJ - 1),
                                    skip_group_check=True)

                def elementwise(h):
                    if skip_ew:
                        return
                    jlo = h * HJ
                    Ecur, Scur, Icur, Pcur = E[h], Sn[h], If[h], Ip[h]
                    Pnext = lp.tile([128, HJ, 2, BL], BF16, tag=f"P{h}", bufs=4)
                    Ifs = []
                    for s in range(2):
                        # E' = (1-sg)*Ecur + S*lamB  (Scur negated)
                        Mh = lp.tile([128, HJ, BL], F32, tag=f"M{h}", bufs=3)
                        nc.vector.tensor_mul(Mh, Scur, ps[h][:, :, s, :])
                        A0 = lp.tile([128, HJ, BL], F32, tag=f"A{h}", bufs=3)
                        nc.scalar.activation(A0, Ecur, Act.Copy, scale=float(c1s))
                        E1 = lp.tile([128, HJ, BL], F32, tag=f"E{h}", bufs=4)
                        nc.vector.tensor_sub(E1, A0, Mh)
                        # S' = max(1 - (E'+I'), .01); stored negated:
                        # Sn' = min(E'+I', .99) - 1
                        C0 = lp.tile([128, HJ, BL], F32, tag=f"C{h}", bufs=3)
                        nc.vector.tensor_add(C0, E1, Icur)
                        S1 = lp.tile([128, HJ, BL], F32, tag=f"S{h}", bufs=4)
                        nc.vector.tensor_scalar(S1, C0, 0.99, 1.0,
                                                Alu.min, Alu.subtract)
                        # I'' = relu((1-g)*I' + sg*E')
                        B0 = lp.tile([128, HJ, BL], F32, tag=f"B{h}", bufs=3)
                        nc.vector.tensor_scalar(B0, E1, float(sg), None,
                                                Alu.mult)
                        Ipre = lp.tile([128, HJ, BL], F32, tag=f"R{h}", bufs=3)
                        nc.scalar.activation(Ipre, Icur, Act.Copy,
                                             scale=float(c1g))
                        Isum = lp.tile([128, HJ, BL], F32, tag=f"U{h}", bufs=3)
                        nc.vector.tensor_add(Isum, Ipre, B0)
                        I2 = lp.tile([128, HJ, BL], F32, tag=f"I{h}", bufs=4)
                        nc.scalar.activation(I2, Isum, Act.Relu)
                        nc.vector.tensor_copy(Pnext[:, :, s, :], I2)
                        if out_dmas:
                            nc.sync.dma_start(
                                out=i_out[t2 + s, :, jlo:jlo + HJ, :], in_=I2)
                        Ecur, Scur, Icur = E1, S1, I2
                        Ifs.append(I2)
                    E[h], Sn[h], If[h], Ip[h] = Ecur, Scur, Ifs[1], Pnext

                sweeps(0)
                elementwise(0)
                sweeps(1)
                elementwise(1)
    nc.finalize()
    return nc


def _get_program(c1s, sg, c1g, beta, npairs=PAIRS, out_dmas=True, skip_ew=False):
    key = (round(c1s, 9), round(sg, 9), round(c1g, 9), round(beta, 9), npairs,
           out_dmas, skip_ew)
    if key not in _cache:
        _cache[key] = _build(c1s, sg, c1g, beta, npairs, out_dmas, skip_ew)
    return _cache[key]


def _run(x_hist, beta, sigma, gamma, pi_logits, trace=False):
    assert x_hist.shape == (B, T, N) and pi_logits.shape == (N, N)
    for v, nm in ((beta, "beta"), (sigma, "sigma"), (gamma, "gamma")):
        assert np.ptp(np.asarray(v)) == 0.0, f"{nm} must be uniform"
    bta = float(np.asarray(beta).flat[0])
    sgm = float(np.asarray(sigma).flat[0])
    gma = float(np.asarray(gamma).flat[0])

    nc = _get_program(1.0 - sgm, sgm, 1.0 - gma, bta)

    lg = np.ascontiguousarray(np.asarray(pi_logits, dtype=np.float32))
    in_maps = []
    for c in range(NCORES):
        x0s = np.asarray(x_hist[c * BL:(c + 1) * BL, 0, :], dtype=np.float32)
        # pack [p, j, b]: x0p[p, j, b] = x0s[b, j*128+p]
        x0p = np.ascontiguousarray(
            np.transpose(x0s.reshape(BL, NJ, 128), (2, 1, 0)))
        in_maps.append({"logits": lg, "x0": x0p})

    res = run_bass_kernel_spmd(nc, in_maps, list(range(NCORES)), trace=trace)
    shards = []
    for c in range(NCORES):
        o = res.results[c]["i_out"]          # [T, 128, NJ, BL]
        # n = j*128 + p  ->  [BL, T, NJ, 128] -> [BL, T, N]
        shards.append(np.transpose(o, (3, 0, 2, 1)).reshape(BL, T, N))
    i_sim = np.ascontiguousarray(np.concatenate(shards, axis=0))
    pi = res.results[0]["pi_out"]
    return (i_sim, pi), res


def kernel(x_hist, beta, sigma, gamma, pi_logits):
    out, _ = _run(x_hist, beta, sigma, gamma, pi_logits)
    return out
